# revision 23
# baseline (speedup 1.0000x reference)
"""Trainium2 Bass kernel for the attention layer:

    f = wf@x+bf; g = wg@x+bg; h = wh@x+bh            (1x1 convs, Ci=32)
    attn = softmax(f^T g, axis=-1)                   (per batch, N=4096)
    out = (wv @ (h @ attn^T) + bv) * gamma + x

Sharding: 8 cores = 4 batches x 2 query-halves (2048 queries each).
Each core receives the full (256, 4096) batch slice with its query half
permuted to the front, so the SPMD program uses fixed offsets.

Key design points (vs. the fp32r baseline):
  - logits are reassociated: f^T g = x_q^T (wf^T wg) x_k. A = wf^T wg is
    a weight-only 256x256 matrix folded on host, so the contraction K
    becomes 256 and maps onto fp8 DoubleRow matmuls (2 k-tiles of 128,
    0.5 cycles/row): g' = A x on device (8 DoubleRow matmuls), then
    logitsT tiles [128k x 512q] at 256 cycles each.  f and g are never
    materialized; the bf/bg bias terms reduce to a per-key logit offset
    u_k = (bf@wg)@x_k (query-side terms cancel in softmax), computed on
    host and folded into the exp bias.
  - x0 = h @ attn^T accumulates in fp8 DoubleRow as well: hT (keys x
    [ones|h]) quantized to e5m2, exp values stored as e5m2.
  - exp is the real bottleneck (~8.4M elements/core, ACT-only would be
    66us), so it is split across THREE engines: ACT does true
    exp->e5m2 (bias AP = per-key offset), while DVE and GPSIMD compute
    the e5m2 BIT PATTERN directly with one affine op each
    (Schraudolph: uint8_saturate(round(5.77*(logit+u-16) + 60)) is the
    e5m2 encoding of ~exp(logit+u-16)).  Work is assigned greedily by
    a static cost model.  The softmax denominator rides along as a
    "ones" column in hT (row 0 of the x0 PSUM tile).
  - residual + projection: v = wv' x0a accumulates in PSUM, then the
    residual x is ADDED BY THE PE (identity matmul accumulate), so the
    vector engines only do one PSUM->SBUF copy per output tile.
  - all global scaling (A*256, h*64, gamma, 1/denominator) is folded
    into host-side weight prep / the exp affine / wv'.
"""

import math
import os
import numpy as np
import ml_dtypes

import concourse.bass as bass
import concourse.mybir as mybir
import concourse.tile as tile
from concourse import bacc
from concourse.bass import ts
from concourse.bass_utils import run_bass_kernel_spmd

F32 = mybir.dt.float32
F32R = mybir.dt.float32r
U8 = mybir.dt.uint8
F8E4 = mybir.dt.float8e4
F8E5 = mybir.dt.float8e5
DR = mybir.MatmulPerfMode.DoubleRow
EXPF = mybir.ActivationFunctionType.Exp
ADD = mybir.AluOpType.add
MULT = mybir.AluOpType.mult

B, C, W, H = 4, 256, 64, 64
N = W * H            # 4096 keys per batch
CI = 32              # inner channels
NCORES = 8
NQ = N // 2          # queries per core
QC = 512             # query chunk (one fp32 PSUM bank)
QP = 2 * QC          # query pair chunk (exp tile width)
NQP = NQ // QP       # 2 query-pair iterations per core
KC = 128             # key chunk = partition dim
NKC = N // KC        # 32 key chunks
MH = 34              # x0 partitions: [denominator ones | 32 h | pad]
MHP = 48             # padded k-tile stride: DoubleRow needs 16B alignment
NWARM = 8
RUNWAY = 2           # x0 pairs lag behind logits by this many pairs
LOGITS_DR = True     # debug toggle: DoubleRow for the logits matmuls
X0_DR = True         # debug toggle: DoubleRow for the x0 matmuls
CONST_BIAS = False   # debug toggle: use float exp biases (assumes bf==0)

SA = 128.0           # A (logits weight) scale: keeps |g'| < 240
SH = 64.0            # h scale
CSHIFT = 16.0        # global exp shift (softmax-invariant)
L2E4 = 4.0 * math.log2(math.e)            # 5.7708: e5m2 codes per nat
AEXP_SCH = L2E4 / SA                      # Schraudolph scale on psum
AEXP_NAT = 1.0 / SA                       # true-exp scale on psum

TRACE = False
DEBUG = False
LAST_EXEC_NS = None
LAST_RES = None

_cached_nc = None


class Sched:
    """Greedy static load balancer for the three elementwise engines."""

    # (ns per free-dim element, fixed ns per instruction).  GPSIMD has no
    # PSUM port (BIR verifier rejects Pool-engine PSUM operands), so all
    # PSUM-sourced elementwise work must go to ACT or DVE.
    COST = {"act": (0.87, 240), "dve": (1.08, 250), "gp": (2.05, 170)}

    def __init__(self):
        self.load = {"act": 0.0, "dve": 0.0, "gp": 0.0}

    def pick(self, nfree, allowed=("act", "dve")):
        def fin(e):
            v, f = self.COST[e]
            return self.load[e] + v * nfree + f
        best = min(allowed, key=fin)
        self.load[best] = fin(best)
        return best

    def charge(self, eng, nfree):
        v, f = self.COST[eng]
        self.load[eng] += v * nfree + f


def _build():
    nc = bacc.Bacc(
        "TRN2", target_bir_lowering=False, debug=False, num_devices=NCORES
    )
    x8_d = nc.dram_tensor("x8", (C, N), U8, kind="ExternalInput").ap()
    x32_d = nc.dram_tensor("x32", (C, N), F32R, kind="ExternalInput").ap()
    ucs_d = nc.dram_tensor("ucs", (128, NKC), F32, kind="ExternalInput").ap()
    ucn_d = nc.dram_tensor("ucn", (128, NKC), F32, kind="ExternalInput").ap()
    A8_d = nc.dram_tensor("A8", (128, 512), U8, kind="ExternalInput").ap()
    whu8_d = nc.dram_tensor("whu8", (128, 2 * MHP), U8, kind="ExternalInput").ap()
    wvp_d = nc.dram_tensor("wvp", (MH, C), F32R, kind="ExternalInput").ap()
    ident_d = nc.dram_tensor("ident", (128, 128), F32R, kind="ExternalInput").ap()
    out_d = nc.dram_tensor("out", (C, NQ), F32, kind="ExternalOutput").ap()
    if DEBUG:
        dbg_g8_d = nc.dram_tensor("dbg_g8", (128, 2 * N), U8, kind="ExternalOutput").ap()
        dbg_hT_d = nc.dram_tensor("dbg_hT", (128, NKC * MHP), U8, kind="ExternalOutput").ap()
        dbg_eT_d = [
            nc.dram_tensor(f"dbg_eT{i}", (128, 2 * QP), U8, kind="ExternalOutput").ap()
            for i in range(4)
        ]
        dbg_x0a_d = nc.dram_tensor("dbg_x0a", (MH, QC), F32R, kind="ExternalOutput").ap()
        dbg_rcp_d = nc.dram_tensor("dbg_rcp", (1, QC), F32, kind="ExternalOutput").ap()
        dbg_uc_d = nc.dram_tensor("dbg_uc", (128, 2 * NKC), F32, kind="ExternalOutput").ap()
        dbg_ps_d = [
            nc.dram_tensor(f"dbg_ps{i}", (128, QP), F32, kind="ExternalOutput").ap()
            for i in range(4)
        ]

    x8r = x8_d.rearrange("(cc p) n -> p cc n", p=128)
    x32r = x32_d.rearrange("(cc p) n -> p cc n", p=128)
    outr = out_d.rearrange("(oc p) n -> p oc n", p=128)

    sched = Sched()

    with tile.TileContext(nc) as tc:
        with (
            tc.tile_pool(name="consts", bufs=1) as consts,
            tc.tile_pool(name="data", bufs=1) as data,
            tc.tile_pool(name="eTp", bufs=8) as eTp,
            tc.tile_pool(name="smallp", bufs=2) as smallp,
            tc.tile_pool(name="outp", bufs=3) as outp,
            tc.tile_pool(name="pl", bufs=3, space="PSUM") as pl,
            tc.tile_pool(name="px0", bufs=2, space="PSUM") as px0,
        ):
            # ---- PE warm-up (overlaps the input DMAs) ----
            scratch = consts.tile([128, QC], F32)
            nc.vector.memset(scratch, 0.0)
            wps = px0.tile([128, QC], F32, tag="x0", name="warm")
            for i in range(NWARM):
                nc.tensor.matmul(
                    wps[:, 0:256], lhsT=scratch[:, 0:128].bitcast(F32R),
                    rhs=scratch[:, 0:256].bitcast(F32R),
                    start=True, stop=True, skip_group_check=True,
                )
            # absorb the ACT exp-table load before the main loop
            scratch2 = consts.tile([1, 8], F8E5)
            nc.scalar.activation(
                out=scratch2, in_=scratch[0:1, 0:8].bitcast(F32), func=EXPF
            )

            # ---- x8 input first (hTu/g'/logits all need it) ----
            x8_sb = data.tile([128, 2, N], U8)
            x32_sb = data.tile([128, 2, N], F32R)
            for s in range(4):
                nc.sync.dma_start(
                    out=x8_sb[:, :, ts(s, N // 4)], in_=x8r[:, :, ts(s, N // 4)]
                )

            # ---- constants ----

            whu8_sb = consts.tile([128, 2, MHP], U8)
            nc.sync.dma_start(
                out=whu8_sb, in_=whu8_d.rearrange("p (cc m) -> p cc m", cc=2)
            )
            A8_sb = consts.tile([128, 2, 2, 128], U8)
            nc.sync.dma_start(
                out=A8_sb, in_=A8_d.rearrange("p (cc o m) -> p cc o m", cc=2, o=2)
            )
            ucs_sb = consts.tile([128, NKC], F32)
            nc.sync.dma_start(out=ucs_sb, in_=ucs_d)
            ucn_sb = consts.tile([128, NKC], F32)
            nc.sync.dma_start(out=ucn_sb, in_=ucn_d)
            wvp_sb = consts.tile([MH, 2, 128], F32R)
            nc.sync.dma_start(
                out=wvp_sb, in_=wvp_d.rearrange("p (oc m) -> p oc m", oc=2)
            )
            ident_sb = consts.tile([128, 128], F32R)
            nc.sync.dma_start(out=ident_sb, in_=ident_d)
            cbias_n = consts.tile([128, 1], F32)
            nc.vector.memset(cbias_n, -CSHIFT)
            cbias_s = consts.tile([128, 1], F32)
            nc.vector.memset(cbias_s, L2E4 * -CSHIFT + 60.0)

            g8_sb = data.tile([128, 2, N], F8E4)
            hT8_sb = data.tile([128, NKC, MHP], F8E5)
            nc.vector.memset(hT8_sb, 0.0)
            nc.vector.memset(hT8_sb[:, :, 0:1], 1.0)

            def ew_copy(dst, src, nfree):
                e = sched.pick(nfree)
                if e == "act":
                    nc.scalar.copy(dst, src)
                else:
                    nc.vector.tensor_copy(dst, src)

            # ---- hT (keys x [ones|64h|pad]) in e5m2, 8 key chunks/bank ----
            def emit_hT(b8):
                ph = px0.tile([128, 8, MHP], F32, tag="x0", name=f"ph{b8}")
                for j in range(8):
                    kc = b8 * 8 + j
                    nc.tensor.matmul(
                        ph[:, j, :],
                        lhsT=x8_sb[:, :, ts(kc, KC)].bitcast(F8E4),
                        rhs=whu8_sb.bitcast(F8E4),
                        start=True, stop=True, perf_mode=DR,
                    )
                ew_copy(
                    hT8_sb[:, b8 * 8 : b8 * 8 + 8, 1 : CI + 1], ph[:, :, 0:CI],
                    8 * CI,
                )

            emit_hT(0)

            # ---- g' = A x (256 x N) in e4m3; chunk 0 up front, the rest
            # streamed into the first query-pair's logits loop ----
            def emit_gp(ch):
                pg = pl.tile([128, 2, QC], F32, tag="lg", name=f"g{ch}")
                for o in range(2):
                    nc.tensor.matmul(
                        pg[:, o, :],
                        lhsT=A8_sb[:, :, o, :].bitcast(F8E4),
                        rhs=x8_sb[:, :, ts(ch, QC)].bitcast(F8E4),
                        start=True, stop=True, perf_mode=DR,
                    )
                for o in range(2):
                    ew_copy(g8_sb[:, o, ts(ch, QC)], pg[:, o, :], QC)

            emit_gp(0)
            emit_gp(1)

            # residual x arrives while the main loop runs (needed ~20us in)
            for s in range(4):
                nc.sync.dma_start(
                    out=x32_sb[:, :, ts(s, N // 4)], in_=x32r[:, :, ts(s, N // 4)]
                )

            if DEBUG:
                nc.sync.dma_start(out=dbg_uc_d[:, 0:NKC], in_=ucs_sb)
                nc.sync.dma_start(out=dbg_uc_d[:, NKC : 2 * NKC], in_=ucn_sb)
                nc.sync.dma_start(
                    out=dbg_g8_d, in_=g8_sb.bitcast(U8).rearrange("p a b -> p (a b)")
                )
                nc.sync.dma_start(
                    out=dbg_hT_d, in_=hT8_sb.bitcast(U8).rearrange("p a b -> p (a b)")
                )

            # ---- main loop over query pairs ----
            pend_v = []  # deferred projection/residual work

            def flush_v():
                while pend_v:
                    qg, x0t = pend_v.pop(0)
                    rcp = smallp.tile([1, QC], F32, tag="rcp", name=f"r{qg}")
                    nc.vector.reciprocal_approx_fast(rcp, x0t[0:1, :])
                    rcp_b = smallp.tile([MH, QC], F32, tag="rcpb", name=f"rb{qg}")
                    nc.gpsimd.partition_broadcast(rcp_b, rcp)
                    sched.charge("dve", QC)
                    sched.charge("gp", QC)
                    x0a = smallp.tile([MH, QC], F32R, tag="x0a", name=f"xa{qg}")
                    nc.vector.tensor_mul(x0a, x0t[0:MH, :], rcp_b)
                    if DEBUG and qg == 0:
                        nc.sync.dma_start(out=dbg_x0a_d, in_=x0a)
                        nc.sync.dma_start(out=dbg_rcp_d, in_=rcp)
                    sched.charge("dve", QC)
                    ot = outp.tile([128, 2, QC], F32, tag="ot", name=f"o{qg}")
                    for oc in range(2):
                        vt = px0.tile([128, QC], F32, tag="x0", name=f"v{qg}_{oc}")
                        nc.tensor.matmul(
                            vt, lhsT=wvp_sb[:, oc, :], rhs=x0a,
                            start=True, stop=False,
                        )
                        nc.tensor.matmul(
                            vt, lhsT=ident_sb,
                            rhs=x32_sb[:, oc, ts(qg, QC)],
                            start=False, stop=True,
                        )
                        ew_copy(ot[:, oc, :], vt, QC)
                    nc.sync.dma_start(out=outr[:, :, ts(qg, QC)], in_=ot)

            for qp in range(NQP):
                x0t = [None, None]
                x0q = []

                def emit_x0(pj, eT, qcs=(0, 1)):
                    for qc in qcs:
                        nc.tensor.matmul(
                            x0t[qc],
                            lhsT=hT8_sb[:, 2 * pj : 2 * pj + 2, :],
                            rhs=eT[:, :, qc, :].bitcast(F8E5),
                            start=(pj == 0), stop=(pj == NKC // 2 - 1),
                            perf_mode=DR,
                        )

                for kc in range(NKC):
                    ps = pl.tile([128, 2, QC], F32, tag="lg", name=f"l{qp}_{kc}")
                    for qc in range(2):
                        if LOGITS_DR:
                            nc.tensor.matmul(
                                ps[:, qc, :],
                                lhsT=g8_sb[:, :, ts(kc, KC)],
                                rhs=x8_sb[:, :, ts(2 * qp + qc, QC)].bitcast(F8E4),
                                start=True, stop=True, perf_mode=DR,
                            )
                        else:
                            for cc in range(2):
                                nc.tensor.matmul(
                                    ps[:, qc, :],
                                    lhsT=g8_sb[:, cc, ts(kc, KC)],
                                    rhs=x8_sb[:, cc, ts(2 * qp + qc, QC)].bitcast(F8E4),
                                    start=(cc == 0), stop=(cc == 1),
                                )
                    j = kc % 2
                    if j == 0:
                        eT = eTp.tile(
                            [128, 2, 2, QC], U8, tag="eT",
                            name=f"e{qp}_{kc // 2}",
                        )
                    eng = sched.pick(QP)
                    eslice = eT[:, j, :, :]
                    if DEBUG and qp == 0 and kc < 4:
                        pst = outp.tile([128, 2, QC], F32, tag="ot", name=f"dps{kc}")
                        nc.vector.tensor_copy(pst, ps)
                        nc.sync.dma_start(
                            out=dbg_ps_d[kc].rearrange("p (a b) -> p a b", a=2),
                            in_=pst,
                        )
                    bias_n = cbias_n if CONST_BIAS else ucn_sb[:, kc : kc + 1]
                    bias_s = cbias_s if CONST_BIAS else ucs_sb[:, kc : kc + 1]
                    if eng == "act":
                        nc.scalar.activation(
                            out=eslice.bitcast(F8E5), in_=ps, func=EXPF,
                            bias=bias_n, scale=AEXP_NAT,
                        )
                    else:
                        nc.vector.tensor_scalar(
                            eslice, ps, AEXP_SCH, bias_s, MULT, ADD,
                        )
                    if j == 1:
                        if DEBUG and qp == 0 and kc < 8:
                            nc.sync.dma_start(
                                out=dbg_eT_d[kc // 2],
                                in_=eT.rearrange("p a b c -> p (a b c)"),
                            )
                        x0q.append((kc // 2, eT))
                    if qp == 0 and kc in (1, 2, 3):
                        emit_hT(kc)
                    if qp == 0 and kc % 4 == 1 and 2 + kc // 4 < 8:
                        emit_gp(2 + kc // 4)
                    # flush deferred v-work from the previous qp into this
                    # qp's logits stream (before the x0t allocations, so the
                    # px0 pool rotation stays deadlock-free)
                    if kc == 3:
                        flush_v()
                    if kc == 4:
                        for qc in range(2):
                            x0t[qc] = px0.tile(
                                [MHP, QC], F32, tag="x0", name=f"x0_{qp}_{qc}"
                            )
                    if len(x0q) > RUNWAY:
                        pj, peT = x0q.pop(0)
                        emit_x0(pj, peT)
                # drain: finish qc0 first so its normalize chain starts early
                for pj, peT in x0q:
                    emit_x0(pj, peT, qcs=(0,))
                if qp == NQP - 1:
                    pend_v.append((2 * qp, x0t[0]))
                    flush_v()
                for pj, peT in x0q:
                    emit_x0(pj, peT, qcs=(1,))
                if qp == NQP - 1:
                    pend_v.append((2 * qp + 1, x0t[1]))
                else:
                    for qc in range(2):
                        pend_v.append((2 * qp + qc, x0t[qc]))
            flush_v()

    nc.compile()
    return nc


def kernel(x, wf, bf, wg, bg, wh, bh, wv, bv, gamma):
    global _cached_nc, LAST_EXEC_NS
    if _cached_nc is None:
        _cached_nc = _build()
    nc = _cached_nc

    # NOTE: device float8e4 is IEEE e4m3 (inf at 0x78, max finite 240),
    # i.e. ml_dtypes.float8_e4m3 -- NOT e4m3fn.
    E4 = ml_dtypes.float8_e4m3

    def q4(v):
        return np.clip(v, -240.0, 240.0).astype(E4).view(np.uint8)

    x = np.asarray(x, dtype=np.float32)
    wf = np.asarray(wf, dtype=np.float32)
    bf = np.asarray(bf, dtype=np.float32)
    wg = np.asarray(wg, dtype=np.float32)
    bg = np.asarray(bg, dtype=np.float32)
    wh = np.asarray(wh, dtype=np.float32)
    bh = np.asarray(bh, dtype=np.float32)
    wv = np.asarray(wv, dtype=np.float32)
    bv = np.asarray(bv, dtype=np.float32)
    g0 = float(np.asarray(gamma, dtype=np.float32).reshape(-1)[0])

    xf = np.ascontiguousarray(x.reshape(B, C, N))

    # logits weight A = wf^T wg, scaled into e4m3 range; laid out as
    # lhsT[p, cc, o, m] = (SA*A)[o*128+m, cc*128+p]
    As = (SA * (wf.T @ wg)).astype(np.float32)
    A8 = q4(np.ascontiguousarray(
        As.reshape(2, 128, 2, 128).transpose(3, 2, 0, 1)
    )).reshape(128, 512)

    # h-projection rhs: [64*wh^T | zero pad], laid out [p, cc, m]
    whu = np.zeros((C, MHP), np.float32)
    whu[:, 0:CI] = SH * wh.T
    whu8 = q4(np.ascontiguousarray(
        whu.reshape(2, 128, MHP).transpose(1, 0, 2)
    )).reshape(128, 2 * MHP)

    # wv' rows: [g0*(bv + wv@bh) | g0*wv^T/64 | 0]
    wvp = np.zeros((MH, C), np.float32)
    wvp[0, :] = g0 * (bv + wv @ bh)
    wvp[1 : CI + 1, :] = (g0 / SH) * wv.T
    wvp = np.ascontiguousarray(wvp)

    ident = np.eye(128, dtype=np.float32)
    uvec = bf @ wg  # per-key logit offset direction: u_k = uvec @ x_k

    in_maps = []
    for core in range(NCORES):
        b, half = divmod(core, 2)
        xb = xf[b]
        if half:
            xb = np.ascontiguousarray(
                np.concatenate([xb[:, NQ:], xb[:, :NQ]], axis=1)
            )
        u = (uvec @ xb).astype(np.float32)          # (N,)
        ut = u.reshape(NKC, 128).T                  # [p, kc]
        ucs = (L2E4 * (ut - CSHIFT) + 60.0).astype(np.float32)
        ucn = (ut - CSHIFT).astype(np.float32)
        in_maps.append({
            "x8": q4(xb),
            "x32": xb,
            "ucs": np.ascontiguousarray(ucs),
            "ucn": np.ascontiguousarray(ucn),
            "A8": A8, "whu8": whu8, "wvp": wvp, "ident": ident,
        })

    res = run_bass_kernel_spmd(
        nc, in_maps, list(range(NCORES)),
        trace=TRACE or bool(os.environ.get("BASS_KERNEL_TRACE")),
    )
    LAST_EXEC_NS = res.exec_time_ns
    global LAST_RES
    LAST_RES = res

    out = np.empty((B, C, N), np.float32)
    for core in range(NCORES):
        b, half = divmod(core, 2)
        out[b][:, half * NQ : (half + 1) * NQ] = res.results[core]["out"]
    return out.reshape(B, C, W, H)
